# revision 4
# baseline (speedup 1.0000x reference)
"""Trainium2 Bass kernel for nn_ExpandEvecs.

Computes, for evecs [B=4, C=1, M=1024, K=32] and max_lvl=16, the stack of
cumulative low-rank reconstructions
    out[b, l] = V[:, :l+1] @ V[:, :l+1]^T      (V = evecs[b, 0, :, :max_lvl])
returned as [B, max_lvl, M, M] float32 (256 MiB) — an output-DMA-bound
problem (~32 MiB written per core across 8 cores, vs a ~435 GB/s per-core
SBUF-port/fabric ceiling => ~77 us of pure streaming per core).

Sharding: core i handles batch b = i//2 and row-half h = i%2 (512 rows of
every level's M x M matrix).

Precision trick: on the host each eigenvector value v is split as
v = H + E with H = fp16(v), E = fp16(v - H); v_l v_l^T ~= H H^T + H E^T +
E H^T elementwise (dropped E E^T is ~2^-22 relative).

Compute trick (v2): cube[l] = cube[l-1] + v_l v_l^T, so each level only
needs a rank-3 matmul (the 3 interleaved component rows of level l)
ACCUMULATED onto a persistent PSUM bank (start=(l==0), stop=True), then a
PSUM->SBUF snapshot copy per level. This cuts TensorE streaming 16x vs
recomputing the full prefix Gram per level: the PE (HAM-cold ~427 ns per
512-col matmul) stops being the pipeline pacer and the kernel becomes
purely output-DMA-paced.

Per level: 8 rank-3 matmuls (one per 512-wide chunk, 8 single-bank PSUM
tiles), 8 PSUM->SBUF copies alternating VectorE/ScalarE, then a 2 MiB
output DMA (128 x 16 KiB descriptors) alternating between the two HWDGE
rings (sync/scalar). The first FINE_LEVELS levels DMA per 512 KiB g-slot
chunk so output bandwidth ramps immediately after the ~3 us framework
preamble. Level blocks of the (tiny) inputs live along the SBUF free dim
([3, blocks]) because matmul operands must start at partition 0/32/64.
"""

import sys

for _p in ("/root/.axon_site/_ro/trn_rl_repo", "/opt/trn_rl_repo"):
    if _p not in sys.path:
        sys.path.insert(0, _p)

import numpy as np

import concourse.bacc as bacc
import concourse.mybir as mybir
from concourse.tile import TileContext
from concourse import bass_utils

B, C, M, K, L = 4, 1, 1024, 32, 16
HALF = M // 2
P = 128
F32 = mybir.dt.float32
F16 = mybir.dt.float16

OUT_BUFS = 8
FINE_LEVELS = 2


def build_nc(out_bufs=OUT_BUFS, fine=FINE_LEVELS):
    nc = bacc.Bacc("TRN2", target_bir_lowering=False, debug=False)
    # [3 components, level-major blocks]: rows blocks of 128 per (l, g),
    # full blocks of 1024 per l. Components: rows=(H,E,H), full=(H,H,E).
    vt3_rows = nc.dram_tensor("vt3_rows", [3, L * 512], F16, kind="ExternalInput")
    vt3_full = nc.dram_tensor("vt3_full", [3, L * 1024], F16, kind="ExternalInput")
    out = nc.dram_tensor("out", [L, HALF, M], F32, kind="ExternalOutput")

    # Partition p carries rows 4p..4p+3 of each level (g = row mod 4), so a
    # level's DMA sees 16 KiB contiguous DRAM per partition.
    out_r = out.ap().rearrange("l (p g) n -> l p g n", g=4)

    with TileContext(nc) as tc:
        with (
            tc.tile_pool(name="consts", bufs=1) as consts,
            tc.tile_pool(name="outp", bufs=out_bufs) as outp,
            tc.tile_pool(name="psum", bufs=1, space="PSUM") as psump,
        ):
            vr = consts.tile([3, L * 512], F16)
            vf = consts.tile([3, L * 1024], F16)
            # level-0 prefix first so the first matmuls start ASAP
            nc.scalar.dma_start(out=vr[:, 0:512], in_=vt3_rows.ap()[:, 0:512])
            nc.sync.dma_start(out=vf[:, 0:1024], in_=vt3_full.ap()[:, 0:1024])
            nc.scalar.dma_start(out=vr[:, 512:], in_=vt3_rows.ap()[:, 512:])
            nc.sync.dma_start(out=vf[:, 1024:], in_=vt3_full.ap()[:, 1024:])

            # one persistent single-bank PSUM tile per 512-wide chunk;
            # rank-3 accumulation across levels happens in place.
            pts = [psump.tile([P, 512], F32, name=f"pt{c}") for c in range(8)]

            cnt = 0
            for l in range(L):
                ot = outp.tile([P, 4096], F32)
                for g in range(4):
                    lhsT = vr[:, (4 * l + g) * 128 : (4 * l + g + 1) * 128]
                    for nch in range(2):
                        c = 2 * g + nch
                        nc.tensor.matmul(
                            pts[c],
                            lhsT,
                            vf[:, 1024 * l + 512 * nch : 1024 * l + 512 * (nch + 1)],
                            start=(l == 0),
                            stop=True,
                        )
                        dst = ot[:, c * 512 : (c + 1) * 512]
                        if cnt % 2 == 0:
                            nc.vector.tensor_copy(out=dst, in_=pts[c])
                        else:
                            nc.scalar.copy(out=dst, in_=pts[c])
                        cnt += 1
                    if l < fine:
                        dma_eng = nc.sync if (l + g) % 2 == 0 else nc.scalar
                        dma_eng.dma_start(
                            out=out_r[l][:, g : g + 1, :],
                            in_=ot[:, g * M : (g + 1) * M].rearrange(
                                "p (g n) -> p g n", g=1
                            ),
                        )
                if l >= fine:
                    dma_eng = nc.sync if l % 2 == 0 else nc.scalar
                    dma_eng.dma_start(
                        out=out_r[l],
                        in_=ot[:, :].rearrange("p (g n) -> p g n", n=M),
                    )
    nc.compile()
    return nc


_NC_CACHE = {}


def _get_nc():
    key = (OUT_BUFS, FINE_LEVELS)
    if key not in _NC_CACHE:
        _NC_CACHE[key] = build_nc(OUT_BUFS, FINE_LEVELS)
    return _NC_CACHE[key]


def make_in_maps(evecs):
    evecs = np.asarray(evecs, dtype=np.float32)
    in_maps = []
    for core in range(8):
        b, h = core // 2, core % 2
        vt = np.ascontiguousarray(evecs[b, 0, :, :L].T)  # [L, M] fp32
        hi = vt.astype(np.float16)
        lo = (vt - hi.astype(np.float32)).astype(np.float16)
        # rhs blocks: per level, (H, H, E) rows of all M columns
        full = np.stack([hi, hi, lo], axis=0).reshape(3, L * M)
        # lhsT blocks: per (level, g), (H, E, H) of rows h*512 + 4p + g
        hr = hi[:, h * HALF : (h + 1) * HALF].reshape(L, P, 4).transpose(0, 2, 1)
        lr = lo[:, h * HALF : (h + 1) * HALF].reshape(L, P, 4).transpose(0, 2, 1)
        rows = np.stack([hr, lr, hr], axis=0).reshape(3, L * 512)
        in_maps.append(
            {
                "vt3_full": np.ascontiguousarray(full),
                "vt3_rows": np.ascontiguousarray(rows),
            }
        )
    return in_maps


def assemble(results):
    full = np.empty((B, L * C, M, M), dtype=np.float32)
    for core in range(8):
        b, h = core // 2, core % 2
        full[b, :, h * HALF : (h + 1) * HALF, :] = results[core]["out"]
    return full


def kernel(evecs, max_lvl):
    assert int(max_lvl) == L, f"kernel hardcodes max_lvl={L}, got {max_lvl}"
    nc = _get_nc()
    res = bass_utils.run_bass_kernel_spmd(nc, make_in_maps(evecs), list(range(8)))
    return assemble(res.results)
